# revision 33
# baseline (speedup 1.0000x reference)
"""GNN message-passing cell (3-step, 6 SpMMs) on 8 Trainium2 NeuronCores.

Strategy: 1D dest-node sharding. Each core owns 6272 rows (49 tiles of 128).
Per SpMM, edges are grouped by (dest core, source arrival-phase) and packed
into a SHARED chunk schedule (scheme C): tile boundaries fall mid-chunk; the
boundary chunk is consumed by TWO matmuls (the outgoing tile's main one-hot,
val-masked, plus a duplicate boundary one-hot masked to the incoming tile).
This removes the per-(tile,core) pad-to-128 that cost ~20-30% of all gathered
indices.  Neighbor features are fetched with dma_gather (1024 idx/call, 4
SWDGE queues) from HBM-replicated bf16 state tables built via chunked
AllGather; one-hot scatter matrices are built per 8-chunk window with two
broadcast tensor_tensor ops on DVE; PSUM tiles are evicted by the idle
Scalar engine (direct fp32 into the accumulator for a fresh state, bf16
staging + one batched DVE add per phase otherwise).

Runtime specialization on the idx tensors (program built after reading them):
 - duplicate-pass elimination (residual == seq[0] recomputes state1);
 - residual hoisting + SAME-STEP merge: two passes on the same adjacency
   (different source states) share ONE gather; both source states are stored
   as one WIDE table row (512B) so a single descriptor feeds both matmuls.
   For seq=[2,2,4]/res=[2,4,5] this yields 4 real passes (from 6).
 - passes within a step are ordered by the newest table they read, so older-
   table passes run while the newest AllGather is still in flight.

Latency hiding: accumulators are split per exchange-chunk (25/24 tiles), and
each state's chunk-0 AllGather fires MID-PASS (2 tiles after the chunk's last
eviction), overlapping the collective with the pass tail; chunk-1 follows at
pass end and hides under the next pass's phase-0 windows.

GpSimd desc-gen (994ns/call + ~2.4ns/idx, 1024-idx max per call — the 16KB
staging is ucode-hardcoded, 2048-idx calls wedge the device) paces the
kernel; the effective ~4-4.6us/call steady state also needs deep g_t
buffering (gp bufs>=6), else the gather->oh->matmul->evict->reuse latency
loop throttles issue (observed as uniform ~9.5us calls on 3 of 4 queues).

Timeline: 2.79ms (staged baseline) -> 1.88ms via: batched one-hot build
(per-window broadcast tensor_tensor instead of per-chunk tensor_scalar —
vector was 93% busy and backpressured everything), scheme-C packing (-11%
idx), same-step merge (-20% idx), scalar-engine PSUM eviction + batched
adds, pass reordering, split-chunk mid-pass exchanges.
"""
import os
import sys

sys.path.insert(0, "/opt/trn_rl_repo")

import numpy as np

# ---------------- problem constants (hardcoded; must match reference) -------
N_STEP = 3
N_NODES = 50000
N_ADJ = 6
NNZ = 800000
DIN = 256
D = 128
LN_EPS = 1e-5

NCORES = 8
P = 128
TPC = 49                  # dest tiles per core
RPC = TPC * P             # 6272 rows per core
NPAD = NCORES * RPC       # 50176 padded rows
C0_T, C1_T = 25, 24       # shard-chunk split in tiles (for 2-chunk AllGather)
C0, C1 = C0_T * P, C1_T * P          # 3200 / 3072 rows per core per chunk
T0, T1 = NCORES * C0, NCORES * C1    # table chunk sizes: 25600 / 24576
WCH = int(os.environ.get("KERNEL_WCH", "8"))    # gather window, 128-edge chunks
                          # (>1024 idxs per dma_gather overflows the 16KB
                          # descriptor staging carveout and wedges the device)
NSWQ = 4                  # SWDGE queues to rotate gathers over
SCRATCH = 16384           # dyn-DMA carveout/partition (ucode hardcodes 16KB)

DT_BF16 = os.environ.get("KERNEL_BF16", "1") == "1"
HOIST = os.environ.get("KERNEL_HOIST", "1") == "1"
DBG_STEPS = int(os.environ.get("KERNEL_STEPS", str(N_STEP)))  # debug bisection

LAST_RESULTS = {}         # test.py introspection (exec_time etc.)


# ---------------- host-side edge preprocessing ------------------------------
def _prep_spmm(rows, cols, vals):
    """Partition/sort/pack one adjacency's edges (scheme C shared schedule).

    Per phase returns padded per-core streams (tidx int16, dloc, val), the
    boundary one-hot duplicate streams (dl2, vl2) and the shared per-tile
    boundary chunks B[t].
    """
    rows = rows.astype(np.int64)
    cols = cols.astype(np.int64)
    dc = rows // RPC
    lr = rows % RPC
    t = lr // P
    dloc = lr % P
    cs = cols // RPC
    ls = cols % RPC
    ph = (ls >= C0).astype(np.int64)
    tidx = np.where(ph == 0, cs * C0 + ls, cs * C1 + (ls - C0))

    out = []
    for p in range(2):
        m = ph == p
        dcp, tp, dlp, tip, vp = dc[m], t[m], dloc[m], tidx[m], vals[m]
        cnt = np.zeros((NCORES, TPC), np.int64)
        np.add.at(cnt, (dcp, tp), 1)

        # shared boundary schedule: tile tt's edges live in chunks
        # (B[tt-1] .. B[tt]); chunk B[tt] is shared with tile tt+1.
        S = np.zeros(NCORES, np.int64)       # next free slot per core
        B = np.zeros(TPC, np.int64)
        starts = np.zeros((NCORES, TPC), np.int64)
        prevB = -1
        for tt_ in range(TPC):
            if tt_ > 0:
                S = np.maximum(S, prevB * P)   # may start IN boundary chunk
            starts[:, tt_] = S
            E = S + cnt[:, tt_]
            b = int(np.max((np.maximum(E, 1) - 1) // P))
            b = max(b, prevB + 1)
            B[tt_] = b
            S = E
            prevB = b
        L = int(B[-1] + 1) * P

        order = np.lexsort((tip, tp, dcp))
        dcs, ts_, dls, tis, vs = dcp[order], tp[order], dlp[order], tip[order], vp[order]
        key = dcs * TPC + ts_
        runpos = np.searchsorted(key, np.arange(NCORES * TPC))
        runend = np.searchsorted(key, np.arange(NCORES * TPC), side="right")

        ti = np.zeros((NCORES, L), np.int16)
        dl = np.zeros((NCORES, L), np.float32)
        vl = np.zeros((NCORES, L), np.float32)
        nb = TPC - 1
        dl2 = np.zeros((NCORES, nb * P), np.float32)
        vl2 = np.zeros((NCORES, nb * P), np.float32)
        for c in range(NCORES):
            for tt_ in range(TPC):
                a0, b0 = runpos[c * TPC + tt_], runend[c * TPC + tt_]
                n = b0 - a0
                if n == 0:
                    continue
                o = starts[c, tt_]
                ti[c, o:o + n] = tis[a0:b0]
                dl[c, o:o + n] = dls[a0:b0]
                vl[c, o:o + n] = vs[a0:b0]
        # boundary duplicate streams + main-val masking
        pos = np.arange(P)
        for c in range(NCORES):
            for i in range(nb):
                b = int(B[i])
                sl = slice(b * P, (b + 1) * P)
                m2 = (b * P + pos) >= starts[c, i + 1]   # tile i+1's slots
                dl2[c, i * P:(i + 1) * P] = np.where(m2, dl[c, sl], 0)
                vl2[c, i * P:(i + 1) * P] = np.where(m2, vl[c, sl], 0)
                vl[c, sl] = np.where(m2, 0, vl[c, sl])
        # host-baked one-hot streams (bit-identical to the on-device bf16
        # build: val rounds to bf16 either way), wrapped [P, chunks*D]
        import ml_dtypes
        np_DT = ml_dtypes.bfloat16 if DT_BF16 else np.float32
        ohm = np.zeros((NCORES, P, (L // P) * D), np_DT)
        oh2m = np.zeros((NCORES, P, nb * D), np_DT)
        for c in range(NCORES):
            nz = np.nonzero(vl[c])[0]
            ohm[c, nz % P, (nz // P) * D + dl[c, nz].astype(np.int64)] = \
                vl[c, nz]
            nz2 = np.nonzero(vl2[c])[0]
            oh2m[c, nz2 % P, (nz2 // P) * D + dl2[c, nz2].astype(np.int64)] \
                = vl2[c, nz2]
        out.append(dict(tidx=ti, ohm=ohm, oh2m=oh2m,
                        L=L, B=[int(x) for x in B]))
    return out


def _wrap_idx(a):  # [L] int16 -> [128, L/16]
    return np.tile(a.reshape(-1, 16).T, (NCORES, 1)).astype(np.int16)


def _wrap_pe(a, np_dt):  # [L] -> [128, L/128] (edge e -> partition e%128)
    return np.ascontiguousarray(a.reshape(-1, P).T.astype(np_dt))


# ---------------- bass program ----------------------------------------------
def _build(meta):
    import concourse.bacc as bacc
    import concourse.mybir as mybir
    import concourse.tile as tile

    f32 = mybir.dt.float32
    i16 = mybir.dt.int16
    DT = mybir.dt.bfloat16 if DT_BF16 else f32
    Alu = mybir.AluOpType
    Act = mybir.ActivationFunctionType

    nc = bacc.Bacc("TRN2", target_bir_lowering=False, debug=False,
                   num_devices=NCORES, num_swdge_queues=NSWQ,
                   dynamic_dma_scratch_size=SCRATCH)

    xt_d = nc.dram_tensor("xt", [DIN, RPC], DT, kind="ExternalInput")
    w0_d = nc.dram_tensor("w0", [P, D], DT, kind="ExternalInput")
    w1_d = nc.dram_tensor("w1", [P, D], DT, kind="ExternalInput")
    brep_d = nc.dram_tensor("brep", [P, D], f32, kind="ExternalInput")
    grep_d = nc.dram_tensor("grep", [P, D], f32, kind="ExternalInput")
    berep_d = nc.dram_tensor("berep", [P, D], f32, kind="ExternalInput")
    idx_d, ohm_d, oh2_d = {}, {}, {}
    NB = TPC - 1
    for (m, p), L in meta["lengths"].items():
        idx_d[(m, p)] = nc.dram_tensor(f"idx_{m}_{p}", [P, L // 16], i16,
                                       kind="ExternalInput")
        ohm_d[(m, p)] = nc.dram_tensor(f"ohm_{m}_{p}", [P, (L // P) * D], DT,
                                       kind="ExternalInput")
        oh2_d[(m, p)] = nc.dram_tensor(f"oh2_{m}_{p}", [P, NB * D], DT,
                                       kind="ExternalInput")
    out_d = nc.dram_tensor("out", [RPC, D], f32, kind="ExternalOutput")

    ts = lambda t: slice(t * D, (t + 1) * D)
    wide_specs = meta["wide_specs"]   # key -> [sigma_col0, sigma_col1]
    sched = meta["sched"]             # step -> [dict(m, consumers, wkey)]
    out_acc = meta["out_acc"]
    use_b = any(acc == "B" for st in sched for pa in st
                for (_, acc) in pa["consumers"])

    with tile.TileContext(nc) as tc:
        with (
            tc.tile_pool(name="const", bufs=1) as cp,
            tc.tile_pool(name="acc", bufs=1) as ap_,
            tc.tile_pool(name="xp", bufs=2) as xp,
            tc.tile_pool(name="gp", bufs=6) as gp,
            tc.tile_pool(name="op", bufs=4) as op_,
            tc.tile_pool(name="o2", bufs=2) as o2p,
            tc.tile_pool(name="ip", bufs=2) as ip,
            tc.tile_pool(name="stg", bufs=2) as sgp,
            tc.tile_pool(name="cst", bufs=1) as csp,
            tc.tile_pool(name="sp", bufs=4) as stp,
            tc.tile_pool(name="ps", bufs=4, space="PSUM") as pp,
            tc.tile_pool(name="dr", bufs=1, space="DRAM") as dp,
        ):
            w0_t = cp.tile([P, D], DT, name="w0t")
            nc.sync.dma_start(w0_t[:], w0_d[:])
            w1_t = cp.tile([P, D], DT, name="w1t")
            nc.sync.dma_start(w1_t[:], w1_d[:])
            brep_t = cp.tile([P, D], f32, name="brept")
            nc.sync.dma_start(brep_t[:], brep_d[:])
            grep_t = cp.tile([P, D], f32, name="grept")
            nc.sync.dma_start(grep_t[:], grep_d[:])
            berep_t = cp.tile([P, D], f32, name="berept")
            nc.sync.dma_start(berep_t[:], berep_d[:])
            eps_t = cp.tile([P, 1], f32, name="epst")
            nc.vector.memset(eps_t[:], LN_EPS)

            CTS = [C0_T, C1_T]
            accA0 = ap_.tile([P, C0_T * D], f32, name="accA0")
            accA1 = ap_.tile([P, C1_T * D], f32, name="accA1")
            acc = {"A": [accA0, accA1]}
            if use_b:
                accB0 = ap_.tile([P, C0_T * D], f32, name="accB0")
                accB1 = ap_.tile([P, C1_T * D], f32, name="accB1")
                acc["B"] = [accB0, accB1]
            first = {"A": [True] * TPC, "B": [True] * TPC}

            def accsl(an, t):
                c = 0 if t < C0_T else 1
                lt = t - c * C0_T
                return acc[an][c][:, lt * D:(lt + 1) * D]

            # tables: sigma -> (tab0, tab1); wide pair tensors by key
            tables = {}
            wide_tensors = {}         # key -> (wtab0, wtab1)
            for key in wide_specs:
                wide_tensors[key] = (
                    dp.tile([T0, 2 * D], DT, name=f"wtab0_{key}",
                            tag=f"wtab0_{key}"),
                    dp.tile([T1, 2 * D], DT, name=f"wtab1_{key}",
                            tag=f"wtab1_{key}"))

            def exchange_chunk(s, c):
                """AllGather chunk c of state s (from acc A) into its table."""
                if c == 0:
                    tables[s] = (
                        dp.tile([T0, D], DT, name=f"tab0_{s}",
                                tag=f"tab0_{s}", addr_space="Shared"),
                        dp.tile([T1, D], DT, name=f"tab1_{s}",
                                tag=f"tab1_{s}", addr_space="Shared"))
                tt = tables[s][c]
                CT = CTS[c]
                CR = C0 if c == 0 else C1
                if DT_BF16:
                    cast_t = csp.tile([P, CT * D], DT, name="castc",
                                      tag="cast")
                    nc.scalar.activation(out=cast_t[:], in_=acc["A"][c][:],
                                         func=Act.Copy, bias=0.0, scale=1.0)
                    src3 = cast_t[:].rearrange("p (t f) -> p t f", f=D)
                else:
                    src3 = acc["A"][c][:].rearrange("p (t f) -> p t f", f=D)
                agi = dp.tile([CR, D], DT, name=f"agi{c}_{s}",
                              tag=f"agi{c}_{s}")
                nc.scalar.dma_start(
                    agi[:].rearrange("(t p) f -> p t f", p=P), src3)
                nc.gpsimd.collective_compute(
                    "AllGather", Alu.bypass,
                    replica_groups=[list(range(NCORES))],
                    ins=[agi[:]], outs=[tt[:]])
                # copy this state's rows into its wide-table column
                for key, sigmas in wide_specs.items():
                    if s in sigmas:
                        col = sigmas.index(s)
                        w_ = wide_tensors[key][c]
                        nc.scalar.dma_start(w_[:, col * D:(col + 1) * D],
                                            tt[:])

            # ---------------- affine: h0 = x @ W + b ----------------
            for t in range(TPC):
                xt0 = xp.tile([P, P], DT, tag="xt0")
                nc.scalar.dma_start(xt0[:], xt_d[0:P, t * P:(t + 1) * P])
                xt1 = xp.tile([P, P], DT, tag="xt1")
                nc.scalar.dma_start(xt1[:], xt_d[P:DIN, t * P:(t + 1) * P])
                ps = pp.tile([P, D], mybir.dt.float32, name="psa", tag="ps0")
                nc.tensor.matmul(out=ps[:], lhsT=xt0[:], rhs=w0_t[:],
                                 start=True, stop=False)
                nc.tensor.matmul(out=ps[:], lhsT=xt1[:], rhs=w1_t[:],
                                 start=False, stop=True)
                nc.vector.tensor_tensor(out=accsl("A", t), in0=ps[:],
                                        in1=brep_t[:], op=Alu.add)
                if t == C0_T + 1:
                    exchange_chunk(0, 0)
            exchange_chunk(0, 1)
            first["A"] = [False] * TPC

            # ---------------- message-passing steps ----------------
            for i, passes in enumerate(sched[:DBG_STEPS]):
                step_accs = {an for pa in passes
                             for (_, an) in pa["consumers"]}
                # keep_accum: a dropped duplicate pass's contribution is the
                # accumulator's current contents -> accumulate on top of it
                if not meta["keep_accum"][i] and "A" in step_accs:
                    first["A"] = [True] * TPC
                for pa in passes:
                    m = pa["m"]
                    cons = pa["consumers"]
                    ncons = len(cons)
                    sig0 = cons[0][0]
                    completing = (pa is passes[-1]) and (i < DBG_STEPS - 1)
                    for p in range(2):
                        L = meta["lengths"][(m, p)]
                        B = meta["groups"][(m, p)]
                        idx_t = ip.tile([P, L // 16], i16, tag="idx")
                        nc.scalar.dma_start(idx_t[:], idx_d[(m, p)][:])
                        # boundary one-hots for the whole phase (host-baked)
                        o2a = o2p.tile([P, NB, D], DT, name="o2a", tag="o2a")
                        nc.sync.dma_start(
                            o2a[:], oh2_d[(m, p)][:].rearrange(
                                "p (w f) -> p w f", f=D))
                        if ncons == 2:
                            tab_ap = wide_tensors[pa["wkey"]][p][:]
                            gelem = 2 * D
                        else:
                            tab_ap = tables[sig0][p][:]
                            gelem = D
                        total_ch = L // P
                        nwin = (total_ch + WCH - 1) // WCH
                        # per-consumer eviction mode for this phase:
                        # fresh acc -> direct fp32 write; else bf16 staging
                        stage_ci = []
                        stg_t = {}
                        claimed = set()   # an -> direct-write claimed this phase
                        for ci, (sig, an) in enumerate(cons):
                            if all(first[an]) and an not in claimed:
                                stage_ci.append(False)
                                claimed.add(an)
                            else:
                                assert all(first[an]) or not any(first[an])
                                stage_ci.append(True)
                                stg_t[ci] = (
                                    sgp.tile([P, C0_T * D], DT,
                                             name=f"stg{ci}a", tag="stg0"),
                                    sgp.tile([P, C1_T * D], DT,
                                             name=f"stg{ci}b", tag="stg1"))
                        cur_t = 0
                        pstiles = [None] * ncons
                        for w in range(nwin):
                            w0c = w * WCH
                            wlen = min(WCH, total_ch - w0c)
                            nidx = wlen * P
                            g_t = gp.tile([P, WCH, gelem], DT,
                                          tag=f"g_w{ncons}",
                                          bufs=8 if ncons == 1 else 6)
                            nc.gpsimd.dma_gather(
                                g_t[:, :wlen, :], tab_ap,
                                idx_t[:, w0c * 8:(w0c + wlen) * 8],
                                nidx, nidx, gelem,
                                queue_num=w % NSWQ)
                            oh = op_.tile([P, WCH, D], DT, tag="oh")
                            # host-baked one-hot window (streamed from HBM)
                            nc.sync.dma_start(
                                oh[:, :wlen, :],
                                ohm_d[(m, p)][:, w0c * D:(w0c + wlen) * D]
                                .rearrange("p (w f) -> p w f", f=D))
                            for kk in range(w0c, w0c + wlen):
                                for ci in range(ncons):
                                    if cur_t == 0 and kk == 0:
                                        pstiles[ci] = pp.tile(
                                            [P, D], mybir.dt.float32,
                                            name="psm", tag=f"ps{ci}")
                                    nc.tensor.matmul(
                                        out=pstiles[ci][:],
                                        lhsT=oh[:, kk - w0c, :],
                                        rhs=g_t[:, kk - w0c,
                                                ci * D:(ci + 1) * D],
                                        start=(cur_t == 0 and kk == 0),
                                        stop=(kk == B[cur_t]))
                                if kk == B[cur_t]:
                                    # evict tile cur_t for all consumers
                                    cch = 0 if cur_t < C0_T else 1
                                    lt = cur_t - cch * C0_T
                                    for ci, (sig, an) in enumerate(cons):
                                        if stage_ci[ci]:
                                            nc.scalar.activation(
                                                out=stg_t[ci][cch][
                                                    :, lt * D:(lt + 1) * D],
                                                in_=pstiles[ci][:],
                                                func=Act.Copy, bias=0.0,
                                                scale=1.0)
                                        else:
                                            nc.scalar.activation(
                                                out=accsl(an, cur_t),
                                                in_=pstiles[ci][:],
                                                func=Act.Copy, bias=0.0,
                                                scale=1.0)
                                    if cur_t in (C0_T - 1, TPC - 1):
                                        # chunk complete (this phase): fold
                                        # staged contributions into the acc
                                        for ci, (sig, an) in enumerate(cons):
                                            if stage_ci[ci]:
                                                nc.vector.tensor_tensor(
                                                    out=acc[an][cch][:],
                                                    in0=acc[an][cch][:],
                                                    in1=stg_t[ci][cch][:],
                                                    op=Alu.add)
                                    if (completing and p == 1
                                            and cur_t == C0_T + 1):
                                        exchange_chunk(i + 1, 0)
                                    if cur_t < TPC - 1:
                                        # boundary: start next tile's psum
                                        # from this chunk via duplicate
                                        # one-hot masked to tile cur_t+1
                                        for ci in range(ncons):
                                            pstiles[ci] = pp.tile(
                                                [P, D], mybir.dt.float32,
                                                name="psm", tag=f"ps{ci}")
                                            nc.tensor.matmul(
                                                out=pstiles[ci][:],
                                                lhsT=o2a[:, cur_t, :],
                                                rhs=g_t[:, kk - w0c,
                                                        ci * D:(ci + 1) * D],
                                                start=True, stop=False)
                                    cur_t += 1
                        # end of phase: flip first flags for direct writers
                        for ci, (sig, an) in enumerate(cons):
                            if not stage_ci[ci]:
                                for tt_ in range(TPC):
                                    first[an][tt_] = False
                    if completing:
                        exchange_chunk(i + 1, 1)
            for an in set(a for st in sched[:DBG_STEPS] for pa in st
                          for (_, a) in pa["consumers"]):
                for t in range(TPC):
                    assert not first[an][t]

            # ---------------- LayerNorm + GELU (batched, per chunk) --------
            for c, CT in enumerate(CTS):
                Y = acc[out_acc][c]
                if use_b and out_acc == "B":
                    xc = acc["A"][c]  # state accum dead by now -> scratch
                else:
                    xc = ap_.tile([P, CT * D], f32, name=f"lnxc{c}",
                                  tag="lnxc")
                s1 = sgp.tile([P, CT * D], DT, name=f"lns1{c}",
                              tag="stg0" if c == 0 else "stg1")
                Y3 = Y[:].rearrange("p (t f) -> p t f", f=D)
                xc3 = xc[:].rearrange("p (t f) -> p t f", f=D)
                s13 = s1[:].rearrange("p (t f) -> p t f", f=D)

                sum_t = stp.tile([P, CT], f32, name=f"lnsum{c}", tag="lnsum")
                nc.vector.reduce_sum(
                    out=sum_t[:].rearrange("p (t o) -> p t o", o=1), in_=Y3,
                    axis=mybir.AxisListType.X)
                mean_t = stp.tile([P, CT], f32, name=f"lnmean{c}",
                                  tag="lnmean")
                nc.vector.tensor_scalar_mul(out=mean_t[:], in0=sum_t[:],
                                            scalar1=1.0 / D)
                nc.vector.tensor_tensor(
                    out=xc3, in0=Y3,
                    in1=mean_t[:].rearrange("p (t o) -> p t o", o=1)
                    .to_broadcast([P, CT, D]), op=Alu.subtract)
                nc.vector.tensor_tensor(out=s1[:], in0=xc[:], in1=xc[:],
                                        op=Alu.mult)
                var_t = stp.tile([P, CT], f32, name=f"lnvar{c}", tag="lnvar")
                nc.vector.reduce_sum(
                    out=var_t[:].rearrange("p (t o) -> p t o", o=1), in_=s13,
                    axis=mybir.AxisListType.X)
                sd_t = stp.tile([P, CT], f32, name=f"lnsd{c}", tag="lnsd")
                nc.scalar.activation(out=sd_t[:], in_=var_t[:], func=Act.Sqrt,
                                     bias=eps_t[:], scale=1.0 / D)
                rstd_t = stp.tile([P, CT], f32, name=f"lnrstd{c}",
                                  tag="lnrstd")
                nc.vector.reciprocal(out=rstd_t[:], in_=sd_t[:])
                nc.vector.tensor_tensor(
                    out=s13, in0=xc3,
                    in1=rstd_t[:].rearrange("p (t o) -> p t o", o=1)
                    .to_broadcast([P, CT, D]), op=Alu.mult)
                nc.vector.tensor_tensor(
                    out=s13, in0=s13,
                    in1=grep_t[:].rearrange("p (c f) -> p c f", c=1)
                    .to_broadcast([P, CT, D]), op=Alu.mult)
                nc.vector.tensor_tensor(
                    out=s13, in0=s13,
                    in1=berep_t[:].rearrange("p (c f) -> p c f", c=1)
                    .to_broadcast([P, CT, D]), op=Alu.add)
                nc.scalar.activation(out=xc[:], in_=s1[:], func=Act.Gelu)
                nc.sync.dma_start(
                    out_d[c * C0:c * C0 + CT * P].rearrange(
                        "(t p) f -> p t f", p=P), xc3)

    nc.compile()
    n_inst = sum(len(b.instructions) for f in nc.m.functions for b in f.blocks)
    print(f"[kernel] instructions: {n_inst}", flush=True)
    return nc


# ---------------- entry point ------------------------------------------------
def kernel(x, adj_rows, adj_cols, adj_vals, idxes_seq, idxes_res, W, b,
           gamma, beta):
    from concourse.bass_utils import run_bass_kernel_spmd

    import ml_dtypes
    np_DT = ml_dtypes.bfloat16 if DT_BF16 else np.float32

    x = np.asarray(x, np.float32)
    W = np.asarray(W, np.float32)
    b = np.asarray(b, np.float32)
    gamma = np.asarray(gamma, np.float32)
    beta = np.asarray(beta, np.float32)
    adj_rows = np.asarray(adj_rows)
    adj_cols = np.asarray(adj_cols)
    adj_vals = np.asarray(adj_vals, np.float32)
    idxes_seq = np.asarray(idxes_seq).astype(np.int64)
    idxes_res = np.asarray(idxes_res).astype(np.int64)

    # ---- schedule builder (runtime-specialized; general for any idxes) ----
    # step spmm lists with duplicate-pass elimination
    keep_accum = [False] * N_STEP
    step_specs = []           # step -> [(a, sigma)]
    off = 0
    for i in range(N_STEP):
        lst = []
        for j in range(i):
            a = int(idxes_res[off + j])
            if i == 1 and j == 0 and a == int(idxes_seq[0]):
                keep_accum[i] = True   # contribution == current accum contents
                continue
            lst.append((a, j))
        lst.append((int(idxes_seq[i]), i))
        off += i
        step_specs.append(lst)

    # passes: residuals first (their tables exist -> overlap with AllGather)
    sched = [[dict(a=a, sigmas=[s], osteps=[i]) for (a, s) in lst]
             for i, lst in enumerate(step_specs)]
    # hoist: merge a LAST-step residual into an earlier same-adjacency pass
    # whose step already has the residual's table (sigma2 <= step)
    if HOIST and DBG_STEPS == N_STEP:
        last = N_STEP - 1
        for pa in list(sched[last]):
            s2 = pa["sigmas"][0]
            if s2 == last:
                continue              # fresh pass, not hoistable
            done = False
            for i1 in range(s2, last):
                for f in sched[i1]:
                    if (f["a"] == pa["a"] and len(f["sigmas"]) == 1
                            and f["sigmas"][0] != s2):
                        f["sigmas"].append(s2)
                        f["osteps"].append(last)
                        sched[last].remove(pa)
                        done = True
                        break
                if done:
                    break

    # same-step merge: two passes in one step with the same adjacency share
    # one wide gather (all source tables exist by the step's start)
    for st in sched:
        by_a = {}
        for pa in list(st):
            if len(pa["sigmas"]) != 1:
                continue
            a = pa["a"]
            if a in by_a and len(by_a[a]["sigmas"]) == 1:
                by_a[a]["sigmas"].append(pa["sigmas"][0])
                by_a[a]["osteps"].append(pa["osteps"][0])
                st.remove(pa)
            else:
                by_a[a] = pa

    # order passes within a step by the newest table they read: passes on
    # older tables first — they can run while the newest exchange is in flight
    for st in sched:
        st.sort(key=lambda pa: max(pa["sigmas"]))

    deferred = any(len(pa["sigmas"]) == 2 for st in sched for pa in st)
    out_acc = "B" if deferred else "A"
    acc_of_step = ["B" if (deferred and i == N_STEP - 1) else "A"
                   for i in range(N_STEP)]

    # wide tables: one per 2-sigma (combined) pass, cols in sigma order
    wide_specs = {}
    for st in sched:
        for pa in st:
            if len(pa["sigmas"]) == 2:
                key = f"{pa['sigmas'][0]}_{pa['sigmas'][1]}"
                wide_specs[key] = list(pa["sigmas"])
                pa["wkey"] = key
            else:
                pa["wkey"] = None

    # assign stream ids + consumers
    for i, st in enumerate(sched):
        for pa in st:
            pa["consumers"] = [(s, acc_of_step[o])
                               for s, o in zip(pa["sigmas"], pa["osteps"])]
    all_passes = [pa for st in sched for pa in st]
    for m, pa in enumerate(all_passes):
        pa["m"] = m

    # host prep per pass
    lengths, groups = {}, {}
    per_core_streams = {}     # (m,p) -> dict arrays per core
    for pa in all_passes:
        phases = _prep_spmm(adj_rows[pa["a"]], adj_cols[pa["a"]],
                            adj_vals[pa["a"]])
        for p in range(2):
            ph = phases[p]
            lengths[(pa["m"], p)] = ph["L"]
            groups[(pa["m"], p)] = ph["B"]
            per_core_streams[(pa["m"], p)] = ph
    tot = sum(lengths.values())
    print(f"[kernel] idxes_seq={idxes_seq.tolist()} "
          f"idxes_res={idxes_res.tolist()} keep_accum={keep_accum}", flush=True)
    print(f"[kernel] sched={[[(pa['a'], pa['sigmas']) for pa in st] for st in sched]}",
          flush=True)
    print(f"[kernel] passes={len(all_passes)} total idx/core={tot} "
          f"({tot / (len(all_passes) * 2 * NNZ / NCORES / 2) * 100:.1f}% of raw)",
          flush=True)

    meta = dict(lengths=lengths, groups=groups, keep_accum=keep_accum,
                sched=[[{k: pa[k] for k in ("m", "consumers", "wkey")}
                        for pa in st] for st in sched],
                wide_specs=wide_specs, out_acc=out_acc)
    nc = _build(meta)

    # per-core inputs
    xpad = np.zeros((NPAD, DIN), np.float32)
    xpad[:N_NODES] = x
    xt_full = np.ascontiguousarray(xpad.T)

    in_maps = []
    for c in range(NCORES):
        im = dict(
            xt=np.ascontiguousarray(
                xt_full[:, c * RPC:(c + 1) * RPC]).astype(np_DT),
            w0=W[:P].astype(np_DT),
            w1=W[P:].astype(np_DT),
            brep=np.tile(b, (P, 1)).astype(np.float32),
            grep=np.tile(gamma, (P, 1)).astype(np.float32),
            berep=np.tile(beta, (P, 1)).astype(np.float32),
        )
        for (m, p), ph in per_core_streams.items():
            im[f"idx_{m}_{p}"] = _wrap_idx(ph["tidx"][c])
            im[f"ohm_{m}_{p}"] = ph["ohm"][c]
            im[f"oh2_{m}_{p}"] = ph["oh2m"][c]
        in_maps.append(im)

    trace = os.environ.get("KERNEL_TRACE", "0") == "1"
    r = run_bass_kernel_spmd(nc, in_maps, core_ids=list(range(NCORES)),
                             trace=trace)
    LAST_RESULTS["r"] = r

    full = np.concatenate([r.results[c]["out"] for c in range(NCORES)], axis=0)
    return np.ascontiguousarray(full[:N_NODES]).astype(np.float32)
